# revision 1
# baseline (speedup 1.0000x reference)
"""Distributed Bass kernel: RMSNorm + multi-head attention + out-proj on 8 TRN2 cores.

Sharding: sequence-parallel. 4096 token-rows (b=2 x n=2048) split 8 ways ->
each core owns 512 tokens of one batch. Cores 0-3 = batch 0, cores 4-7 =
batch 1. Each core RMSNorms + QKV-projects its tokens, AllGathers K^T and V
within its batch group of 4, runs attention for all 16 heads over its 512
queries, and projects the output locally (no reduce needed: all heads local).
Host concatenates the 8 [512, 1024] output shards.

Layouts: q/k feature-major (qT [dh, tok], kT [dh, tok]) so QK^T needs no
transpose; sim is q-major [q, keys] so softmax stats are free-axis /
per-partition; attn is PE-transposed per 128x128 tile for the AV matmul
(AV col-packed two heads per PE pass via tile_position). The QK path runs
in fp16 (1 cyc/row, ~TF32 mantissa; bf16 would inject ~0.3 abs error into
sim, which softmax amplifies); V/attn are bf16; softmax math is f32.
The softmax shift is the exact row max on the Vector engine (per-chunk
reduces, bufs=6 PSUM pipeline); exp+rowsum run as one ScalarE pass per
chunk via activation(Exp, bias=-max, accum_out). K^T and V are AllGathered
in 4 head-group chunks each, interleaved with the projection matmuls so
comms overlap compute.
"""

import sys

sys.path.insert(0, "/opt/trn_rl_repo")

import numpy as np
import ml_dtypes

import concourse.bass as bass
import concourse.mybir as mybir
import concourse.tile as tile
from concourse import bacc
from concourse.bass_utils import run_bass_kernel_spmd
from concourse.masks import make_identity

F32 = mybir.dt.float32
F32R = mybir.dt.float32r
BF16 = mybir.dt.bfloat16
AF = mybir.ActivationFunctionType
ALU = mybir.AluOpType

B, N, D = 2, 2048, 1024
H, DH = 16, 64
EPS = 1e-5
NC_TOTAL = 8
GROUP = 4           # cores per batch group
TOK = 512           # tokens per core
QT = TOK // 128     # 4 q-tiles per core
KC = N // 512       # 4 key chunks of 512
KC128 = N // 128    # 16 key chunks of 128
DC = D // 128       # 8 contraction chunks

# Matmul input dtype for the sim-critical path (q/k). fp16 streams at
# 1 cyc/row with a 10-bit mantissa (~TF32): sim abs err ~0.05, safe for
# softmax. Half the SBUF footprint and AllGather bytes of f32.
FP16 = mybir.dt.float16
QK_DT = FP16
QK_NP = FP16
# Heads with h % 3 == 1 compute the softmax shift as an exact row-max on
# the Vector engine; the rest use the exp(sim/8)-sum LSE bound on ScalarE.
# Interleaving (rather than a prefix split) keeps both engines loaded
# concurrently through the whole attention phase.
def use_dve_stats(h):
    return True


def mmcast(ap):
    return ap


def build_graph():
    nc = bacc.Bacc(name="attn8")
    x_d = nc.dram_tensor("x", [TOK, D], F32, kind="ExternalInput")
    wqkv_d = nc.dram_tensor("w_qkv", [D, 3 * D], QK_NP, kind="ExternalInput")
    wout_d = nc.dram_tensor("w_out", [D, D], BF16, kind="ExternalInput")
    out_d = nc.dram_tensor("out", [TOK, D], F32, kind="ExternalOutput")

    rg = [list(range(GROUP)), list(range(GROUP, 2 * GROUP))]

    with tile.TileContext(nc) as tc:
        with (
            tc.tile_pool(name="const", bufs=1) as constp,
            tc.tile_pool(name="qt_sb", bufs=H) as qtp,
            tc.tile_pool(name="aoutT", bufs=H // 2) as aoutp,
            tc.tile_pool(name="stats", bufs=16) as statsp,
            tc.tile_pool(name="dram", bufs=1, space="DRAM") as dramp,
        ):
            ident = constp.tile([128, 128], BF16, name="ident")
            make_identity(nc, ident[:])
            epsb = constp.tile([128, 1], F32, name="epsb")
            nc.any.memset(epsb[:], EPS)
            identf = constp.tile([128, 128], QK_NP, name="identf")
            make_identity(nc, identf[:])

            # DRAM bounce buffers for the chunked K^T / V AllGathers:
            # one pair per head-group of 4 heads so attention on group g can
            # start as soon as its two collectives land.
            NG = 4
            bk_in = [dramp.tile([256, TOK], QK_NP, name=f"bk_in{g}")
                     for g in range(NG)]
            bk_out = [dramp.tile([GROUP * 256, TOK], QK_NP, name=f"bk_out{g}")
                      for g in range(NG)]
            bv_in = [dramp.tile([TOK, 256], BF16, name=f"bv_in{g}")
                     for g in range(NG)]
            bv_out = [dramp.tile([GROUP * TOK, 256], BF16, name=f"bv_out{g}")
                      for g in range(NG)]

            qT = [qtp.tile([64, TOK], QK_NP, name=f"qT{h}", tag="qT")
                  for h in range(H)]
            aoutT = [aoutp.tile([128, TOK], BF16, name=f"aoutT{hp}", tag="aT")
                     for hp in range(H // 2)]

            # ---------------- Phase A+B: norm, transpose, QKV ----------------
            with (
                tc.tile_pool(name="xload", bufs=4) as xp,
                tc.tile_pool(name="xnorm", bufs=4) as xnp,
                tc.tile_pool(name="xnT", bufs=DC) as xntp,
                tc.tile_pool(name="wqkv", bufs=DC) as wp,
                tc.tile_pool(name="stage", bufs=2) as stp,
                tc.tile_pool(name="ps_a", bufs=2, space="PSUM") as psa,
                tc.tile_pool(name="ps_b", bufs=2, space="PSUM") as psb,
            ):
                # RMSNorm per q-tile, keep xn in f32 for precision.
                # x loads are queued before the (3x larger) w_qkv load so the
                # norm + transpose pipeline starts immediately.
                xt_tiles = []
                for t in range(QT):
                    xt = xp.tile([128, D], F32, name=f"x{t}", tag="x")
                    nc.sync.dma_start(xt[:], x_d[t * 128 : (t + 1) * 128, :])
                    xt_tiles.append(xt)

                w_sb = []
                for dc in range(DC):
                    w = wp.tile([128, 3 * D], QK_NP, name=f"w{dc}", tag="w")
                    nc.sync.dma_start(w[:], wqkv_d[dc * 128 : (dc + 1) * 128, :])
                    w_sb.append(w)

                xn_t = []
                for t in range(QT):
                    xt = xt_tiles[t]
                    sq = stp.tile([128, D], F32, name=f"sq{t}", tag="sq")
                    ssq = statsp.tile([128, 1], F32, name=f"ssq{t}", tag="st")
                    nc.scalar.activation(sq[:], xt[:], AF.Square,
                                         accum_out=ssq[:])
                    std = statsp.tile([128, 1], F32, name=f"std{t}", tag="st")
                    nc.scalar.activation(std[:], ssq[:], AF.Sqrt,
                                         scale=1.0 / D, bias=epsb[:])
                    rinv = statsp.tile([128, 1], F32, name=f"ri{t}", tag="st")
                    nc.vector.reciprocal(rinv[:], std[:])
                    xn = xnp.tile([128, D], QK_NP, name=f"xn{t}", tag="xn")
                    nc.vector.tensor_scalar_mul(xn[:], xt[:], rinv[:])
                    xn_t.append(xn)

                # transpose xn -> xnT [128 d, 512 tok] x8 (f32)
                xnT = []
                for dc in range(DC):
                    tp = psa.tile([128, 512], QK_NP, name=f"tp{dc}", tag="tpa")
                    for t in range(QT):
                        nc.tensor.transpose(
                            tp[:, t * 128 : (t + 1) * 128],
                            xn_t[t][:, dc * 128 : (dc + 1) * 128],
                            identf[:],
                        )
                    xt2 = xntp.tile([128, TOK], QK_NP, name=f"xnT{dc}",
                                    tag="xnT")
                    nc.scalar.copy(xt2[:], tp[:])
                    xnT.append(xt2)

                def k_proj(fc):
                    # kT feature chunk fc (heads 2fc, 2fc+1) -> bk_in[fc//2]
                    ps = psb.tile([128, TOK], F32, name=f"psk{fc}", tag="psk")
                    for dc in range(DC):
                        nc.tensor.matmul(
                            ps[:],
                            mmcast(w_sb[dc][:, D + fc * 128 : D + (fc + 1) * 128]),
                            mmcast(xnT[dc][:]),
                            start=(dc == 0), stop=(dc == DC - 1),
                        )
                    ksb = stp.tile([128, TOK], QK_NP, name=f"ksb{fc}", tag="ksb")
                    nc.scalar.copy(ksb[:], ps[:])
                    nc.sync.dma_start(
                        bk_in[fc // 2][(fc % 2) * 128 : (fc % 2) * 128 + 128, :],
                        ksb[:])

                def v_proj(vc):
                    # v cols [vc*512, (vc+1)*512) (head groups 2vc, 2vc+1)
                    for t in range(QT):
                        ps = psb.tile([128, 512], F32, name=f"psv{t}{vc}",
                                      tag="psk")
                        for dc in range(DC):
                            nc.tensor.matmul(
                                ps[:],
                                mmcast(xnT[dc][:, t * 128 : (t + 1) * 128]),
                                mmcast(w_sb[dc][:, 2 * D + vc * 512 : 2 * D + (vc + 1) * 512]),
                                start=(dc == 0), stop=(dc == DC - 1),
                            )
                        vsb = stp.tile([128, 512], BF16, name=f"vsb{t}{vc}",
                                       tag="vsb")
                        nc.vector.tensor_copy(vsb[:], ps[:])
                        nc.sync.dma_start(
                            bv_in[2 * vc][t * 128 : (t + 1) * 128, :],
                            vsb[:, 0:256])
                        nc.sync.dma_start(
                            bv_in[2 * vc + 1][t * 128 : (t + 1) * 128, :],
                            vsb[:, 256:512])

                import os as _os
                _fake = _os.environ.get("KERNEL_FAKE_COMM") == "1"

                def ag_k(g):
                    if _fake:
                        nc.sync.dma_start(bk_out[g][0:256, :], bk_in[g][:])
                        return
                    nc.gpsimd.collective_compute(
                        "AllGather", ALU.bypass, replica_groups=rg,
                        ins=[bk_in[g][:].opt()], outs=[bk_out[g][:].opt()])

                def ag_v(g):
                    if _fake:
                        nc.sync.dma_start(bv_out[g][0:TOK, :], bv_in[g][:])
                        return
                    nc.gpsimd.collective_compute(
                        "AllGather", ALU.bypass, replica_groups=rg,
                        ins=[bv_in[g][:].opt()], outs=[bv_out[g][:].opt()])

                def q_proj(h):
                    # qT per head [64, 512] (x8 scale folded into w_q on host)
                    ps = psb.tile([64, TOK], F32, name=f"psq{h}", tag="psq")
                    for dc in range(DC):
                        nc.tensor.matmul(
                            ps[:],
                            mmcast(w_sb[dc][:, h * 64 : (h + 1) * 64]),
                            mmcast(xnT[dc][:]),
                            start=(dc == 0), stop=(dc == DC - 1),
                        )
                    nc.scalar.copy(qT[h][:], ps[:])

                # order: get group 0/1's K and V on the wire as early as
                # possible; later groups' projections overlap earlier comms.
                # The first pair's qT is hoisted so attention starts the
                # moment AG_0 lands.
                k_proj(0); k_proj(1); ag_k(0)
                v_proj(0); ag_v(0)
                q_proj(0); q_proj(1)
                k_proj(2); k_proj(3); ag_k(1); ag_v(1)
                q_proj(2); q_proj(3)
                k_proj(4); k_proj(5); ag_k(2)
                v_proj(1); ag_v(2)
                k_proj(6); k_proj(7); ag_k(3); ag_v(3)
                for h in range(4, H):
                    q_proj(h)

            # ---------------- Phase C: attention ----------------
            # gathered views per group: head h -> group h//4, local i = h%4
            bk_r = [bk_out[g][:].rearrange("(rb f) t -> f rb t", rb=GROUP)
                    for g in range(NG)]
            bv_r = [bv_out[g][:].rearrange("(kc p) e -> p kc e", p=128)
                    for g in range(NG)]

            with (
                tc.tile_pool(name="wout", bufs=H // 2) as woutp,
                tc.tile_pool(name="osb", bufs=2) as osbp,
            ):
                # w_out -> SBUF bf16 [128, 1024] x8, one per head pair
                wout_sb = []
                for hp in range(H // 2):
                    w = woutp.tile([128, D], BF16, name=f"wout{hp}", tag="wout")
                    nc.sync.dma_start(w[:], wout_d[hp * 128 : (hp + 1) * 128, :])
                    wout_sb.append(w)

                with (
                    tc.tile_pool(name="kvh", bufs=4) as kvp,
                    tc.tile_pool(name="attn", bufs=4 * QT) as attnp,
                    tc.tile_pool(name="attnT", bufs=5) as attntp,
                    tc.tile_pool(name="scr", bufs=3) as scrp,
                    tc.tile_pool(name="ps_sim", bufs=5, space="PSUM") as ps_sim,
                    tc.tile_pool(name="ps_xp", bufs=1, space="PSUM") as ps_xp,
                    tc.tile_pool(name="ps_av", bufs=2, space="PSUM") as ps_av,
                ):
                    def load_kv(h):
                        g, hi = divmod(h, 4)
                        kTh = kvp.tile([64, N], QK_NP, name=f"kT{h}", tag="kTh")
                        nc.sync.dma_start(
                            kTh[:].rearrange("f (rb t) -> f rb t", rb=GROUP),
                            bk_r[g][hi * 64 : (hi + 1) * 64])
                        vh = kvp.tile([128, KC128 * 64], BF16, name=f"v{h}",
                                      tag="vh")
                        nc.sync.dma_start(
                            vh[:].rearrange("p (kc e) -> p kc e", kc=KC128),
                            bv_r[g][:, :, hi * 64 : (hi + 1) * 64])
                        return kTh, vh

                    def softmax_head(h, kTh):
                        attn_q = []
                        for t in range(QT):
                            # sim as 4 independent 1-bank chunks so freed
                            # chunks host the next unit's QK immediately
                            simc = []
                            for kc in range(KC):
                                sc = ps_sim.tile([128, 512], F32,
                                                 name=f"sim{h}{t}{kc}",
                                                 tag="sim")
                                nc.tensor.matmul(
                                    sc[:],
                                    mmcast(qT[h][:, t * 128 : (t + 1) * 128]),
                                    mmcast(kTh[:, kc * 512 : (kc + 1) * 512]),
                                    start=True, stop=True)
                                simc.append(sc)
                            negm = statsp.tile([128, 1], F32, name=f"nm{h}{t}",
                                               tag="st")
                            if use_dve_stats(h):
                                # exact row max on DVE (per chunk, combined)
                                pmax = statsp.tile([128, KC], F32,
                                                   name=f"pm{h}{t}", tag="st4")
                                for kc in range(KC):
                                    nc.vector.tensor_reduce(
                                        pmax[:, kc : kc + 1], simc[kc][:],
                                        axis=mybir.AxisListType.X, op=ALU.max)
                                nc.vector.tensor_reduce(
                                    negm[:], pmax[:],
                                    axis=mybir.AxisListType.X,
                                    op=ALU.max, negate=True)
                            else:
                                # pass A: S = sum(exp(sim/8)) on ScalarE;
                                # m = 8*ln2*(exponent(S)-127) ~ 8*ln(S) is a
                                # safe shift in (rowmax-5.6, rowmax+61]
                                stA = statsp.tile([128, KC], F32,
                                                  name=f"sA{h}{t}", tag="st4")
                                for kc in range(KC):
                                    scr = scrp.tile([128, 512], BF16,
                                                    name=f"scr{h}{t}{kc}",
                                                    tag="scr")
                                    nc.scalar.activation(
                                        scr[:], simc[kc][:],
                                        AF.Exp, scale=0.125,
                                        accum_out=stA[:, kc : kc + 1])
                                sS = statsp.tile([128, 1], F32,
                                                 name=f"sS{h}{t}", tag="st")
                                nc.vector.tensor_reduce(
                                    sS[:], stA[:], axis=mybir.AxisListType.X,
                                    op=ALU.add)
                                sh = statsp.tile([128, 1], mybir.dt.int32,
                                                 name=f"sh{h}{t}", tag="sti")
                                nc.vector.tensor_scalar(
                                    sh[:], sS[:].bitcast(mybir.dt.int32), 23,
                                    None, op0=ALU.logical_shift_right)
                                shf = statsp.tile([128, 1], F32,
                                                  name=f"shf{h}{t}", tag="st")
                                nc.vector.tensor_copy(shf[:], sh[:])
                                LN2_8 = 5.545177444479562
                                nc.vector.tensor_scalar(
                                    negm[:], shf[:], -LN2_8, 127.0 * LN2_8,
                                    op0=ALU.mult, op1=ALU.add)
                            # pass B: attn = exp(sim - m), s = rowsum
                            at = attnp.tile([128, N], BF16, name=f"at{h}{t}",
                                            tag="attn")
                            stB = statsp.tile([128, KC], F32, name=f"sB{h}{t}",
                                              tag="st4")
                            for kc in range(KC):
                                nc.scalar.activation(
                                    at[:, kc * 512 : (kc + 1) * 512],
                                    simc[kc][:],
                                    AF.Exp, bias=negm[:],
                                    accum_out=stB[:, kc : kc + 1])
                            s = statsp.tile([128, 1], F32, name=f"s{h}{t}",
                                            tag="st")
                            nc.vector.tensor_reduce(s[:], stB[:],
                                                    axis=mybir.AxisListType.X,
                                                    op=ALU.add)
                            rs = statsp.tile([128, 1], F32, name=f"rs{h}{t}",
                                             tag="st")
                            nc.vector.reciprocal(rs[:], s[:])
                            nc.vector.tensor_scalar_mul(at[:], at[:], rs[:])
                            attn_q.append(at)
                        return attn_q

                    for hp in range(H // 2):
                        h0, h1 = 2 * hp, 2 * hp + 1
                        kv0 = load_kv(h0)
                        kv1 = load_kv(h1)
                        attns = [softmax_head(h0, kv0[0]),
                                 softmax_head(h1, kv1[0])]
                        vhs = [kv0[1], kv1[1]]
                        # transpose attn tiles; AV col-packed: head h0 on PE
                        # cols 0-63 -> av[0:64], h1 on cols 64-127 -> av[64:]
                        av = ps_av.tile([128, TOK], F32, name=f"av{hp}",
                                        tag="av")
                        for kp in range(KC128 // 2):
                            for hh in range(2):
                                h = 2 * hp + hh
                                xpt = ps_xp.tile([128, 2 * TOK], BF16,
                                                 name=f"xp{h}{kp}", tag="xp")
                                for j in range(2):
                                    kc = 2 * kp + j
                                    for t in range(QT):
                                        nc.tensor.transpose(
                                            xpt[:, j * 512 + t * 128 : j * 512 + (t + 1) * 128],
                                            attns[hh][t][:, kc * 128 : (kc + 1) * 128],
                                            ident[:])
                                atT = attntp.tile([128, 2 * TOK], BF16,
                                                  name=f"atT{h}{kp}", tag="atT")
                                # split the PSUM->SBUF copies across both
                                # engines: DVE for h0, ScalarE for h1
                                if hh == 0:
                                    nc.vector.tensor_copy(atT[:], xpt[:])
                                else:
                                    nc.scalar.copy(atT[:], xpt[:])
                                for j in range(2):
                                    kc = 2 * kp + j
                                    nc.tensor.matmul(
                                        av[hh * 64 : hh * 64 + 64, :],
                                        vhs[hh][:, kc * 64 : (kc + 1) * 64],
                                        atT[:, j * 512 : (j + 1) * 512],
                                        start=(kc == 0), stop=(kc == KC128 - 1),
                                        tile_position=(0, 64 * hh))

                        nc.vector.tensor_copy(aoutT[hp][:], av[:])

                # ---------------- Phase D: output projection ----------------
                with tc.tile_pool(name="ps_o", bufs=2, space="PSUM") as pso:
                    for t in range(QT):
                        ot = osbp.tile([128, D], F32, name=f"o{t}", tag="o")
                        for oc in range(2):
                            ps = pso.tile([128, 512], F32, name=f"pso{t}{oc}",
                                          tag="pso")
                            for hp in range(H // 2):
                                nc.tensor.matmul(
                                    ps[:],
                                    aoutT[hp][:, t * 128 : (t + 1) * 128],
                                    wout_sb[hp][:, oc * 512 : (oc + 1) * 512],
                                    start=(hp == 0), stop=(hp == H // 2 - 1))
                            nc.scalar.copy(ot[:, oc * 512 : (oc + 1) * 512],
                                           ps[:])
                        nc.sync.dma_start(out_d[t * 128 : (t + 1) * 128, :],
                                          ot[:])

    nc.finalize()
    return nc


_NC_CACHE = None


def kernel(x, mask, gamma, w_qkv, w_out):
    global _NC_CACHE
    x = np.asarray(x, dtype=np.float32)
    gamma = np.asarray(gamma, dtype=np.float32)
    w_qkv = np.asarray(w_qkv, dtype=np.float32)
    w_out = np.asarray(w_out, dtype=np.float32)

    # fold gamma (RMSNorm scale) and the x8 q-scale into w_qkv (exact in f32)
    w = w_qkv * gamma[:, None]
    w = np.concatenate([w[:, :D] * (DH ** 0.5), w[:, D:]], axis=1)
    w = np.ascontiguousarray(w, dtype=np.float16)
    wo = np.ascontiguousarray(w_out.astype(ml_dtypes.bfloat16))

    if _NC_CACHE is None:
        _NC_CACHE = build_graph()
    nc = _NC_CACHE

    in_maps = []
    for c in range(NC_TOTAL):
        g, r = divmod(c, GROUP)
        xs = np.ascontiguousarray(
            x[g, r * TOK : (r + 1) * TOK, :], dtype=np.float32)
        in_maps.append({"x": xs, "w_qkv": w, "w_out": wo})

    res = run_bass_kernel_spmd(nc, in_maps, core_ids=list(range(NC_TOTAL)))
    out = np.empty((B, N, D), dtype=np.float32)
    for c in range(NC_TOTAL):
        g, r = divmod(c, GROUP)
        out[g, r * TOK : (r + 1) * TOK, :] = res.results[c]["out"]
    return out



# revision 12
# speedup vs baseline: 1.0084x; 1.0084x over previous
"""Distributed Bass kernel: RMSNorm + multi-head attention + out-proj on 8 TRN2 cores.

Sharding: head x batch tensor parallel. Core c owns batch c//4 and heads
[4*(c%4), 4*(c%4)+4) for the full 2048-token sequence. Each core RMSNorms the
whole batch, projects Q/K/V for only its 4 heads (w_qkv column shard), runs
full attention for those heads, and computes a partial output projection
(w_out row shard). A single bf16 ReduceScatter per token-half sums the 4
partials of each batch group and scatters 512 rows back to each core - the
only collective in the kernel (the baseline's 8 serialized K/V AllGathers
cost ~330us on the collective cores).

Attention pipeline per (head, 128-query tile): q-major sim on the PE
(fp16, x8 scale folded into w_q), exact row-max via DVE+Pool psum reduces,
one ScalarE exp pass (bias = -rowmax), DMA-xbar transpose of the bf16 attn
tile into keys-major layout, then a full-PE AV matmul (lhsT = attnT tile,
128x128 stationary; moving operand = [v | ones], 65 columns) whose extra
ones-column yields the softmax denominator for free. Normalization happens
on the tiny [128, 64] AV output, not the [128, 2048] attn matrix.
"""

import sys

sys.path.insert(0, "/opt/trn_rl_repo")

import numpy as np
import ml_dtypes

import concourse.bass as bass
import concourse.mybir as mybir
import concourse.tile as tile
from concourse import bacc
from concourse.bass_utils import run_bass_kernel_spmd
from concourse.masks import make_identity

F32 = mybir.dt.float32
F16 = mybir.dt.float16
BF16 = mybir.dt.bfloat16
AF = mybir.ActivationFunctionType
ALU = mybir.AluOpType

B, N, D = 2, 2048, 1024
H, DH = 16, 64
EPS = 1e-5
NC_TOTAL = 8
HPC = 4                 # heads per core
GROUP = 4               # cores per batch (reduce-scatter group)
NT = N // 128           # 16 token tiles
QT = NT                 # query tiles
KC = NT                 # key chunks of 128
DC = D // 128           # 8 contraction chunks
WQKV_COLS = 3 * HPC * DH  # 768


def build_graph():
    nc = bacc.Bacc(name="attn8")
    x_d = nc.dram_tensor("x", [N, D], F16, kind="ExternalInput")
    w_d = nc.dram_tensor("w_qkv", [D, WQKV_COLS], F16, kind="ExternalInput")
    wout_d = nc.dram_tensor("w_out", [HPC * DH, D], BF16, kind="ExternalInput")
    outp_d = nc.dram_tensor("outp", [N, D], BF16, kind="Internal")
    rsout_d = nc.dram_tensor("rsout", [N // GROUP, D], BF16, kind="Internal")
    out_d = nc.dram_tensor("out", [N // GROUP, D], BF16,
                           kind="ExternalOutput")  # [512, 1024]

    rg = [list(range(GROUP)), list(range(GROUP, 2 * GROUP))]

    with tile.TileContext(nc) as tc:
        with (
            tc.tile_pool(name="const", bufs=1) as constp,
            tc.tile_pool(name="xload", bufs=4) as xp,
            tc.tile_pool(name="xnorm", bufs=NT) as xnp,
            tc.tile_pool(name="xnT", bufs=DC) as xntp,
            tc.tile_pool(name="wqkv", bufs=DC) as wp,
            tc.tile_pool(name="wout", bufs=2) as woutp,
            tc.tile_pool(name="kq", bufs=2) as kqp,
            tc.tile_pool(name="vx", bufs=HPC) as vxp,
            tc.tile_pool(name="stats", bufs=8) as statsp,
            tc.tile_pool(name="scr", bufs=2) as scrp,
            tc.tile_pool(name="attn", bufs=3) as attnp,
            tc.tile_pool(name="attnT", bufs=3) as attntp,
            tc.tile_pool(name="aout", bufs=2 * QT) as aoutp,
            tc.tile_pool(name="aoutT", bufs=2) as aouttp,
            tc.tile_pool(name="osb", bufs=3) as osbp,
            tc.tile_pool(name="ps_a", bufs=3, space="PSUM") as psa,
            tc.tile_pool(name="ps_b", bufs=2, space="PSUM") as psb,
        ):
            identf = constp.tile([128, 128], F16, name="identf")
            make_identity(nc, identf[:])
            identb = constp.tile([128, 128], BF16, name="identb")
            make_identity(nc, identb[:])
            epsb = constp.tile([128, 1], F32, name="epsb")
            nc.any.memset(epsb[:], EPS)

            # ---------------- DMA loads ----------------
            xt = []
            for t in range(NT):
                xl = xp.tile([128, D], F16, name=f"x{t}", tag="x")
                nc.sync.dma_start(xl[:], x_d[t * 128:(t + 1) * 128, :])
                xt.append(xl)
            w_sb = []
            for dc in range(DC):
                w = wp.tile([128, WQKV_COLS], F16, name=f"w{dc}", tag="w")
                nc.sync.dma_start(w[:], w_d[dc * 128:(dc + 1) * 128, :])
                w_sb.append(w)
            wout_sb = []
            for i in range(2):
                w = woutp.tile([128, D], BF16, name=f"wo{i}", tag="wo")
                nc.sync.dma_start(w[:], wout_d[i * 128:(i + 1) * 128, :])
                wout_sb.append(w)

            # ---------------- RMSNorm (per token tile) ----------------
            xn = []
            for t in range(NT):
                scr = scrp.tile([128, D], F16, name=f"scr{t}", tag="scr")
                ssq = statsp.tile([128, 1], F32, name=f"ssq{t}", tag="ssq")
                nc.scalar.activation(scr[:], xt[t][:], AF.Square,
                                     accum_out=ssq[:])
                std = statsp.tile([128, 1], F32, name=f"std{t}", tag="ssq")
                nc.scalar.activation(std[:], ssq[:], AF.Sqrt, scale=1.0 / D,
                                     bias=epsb[:])
                rinv = statsp.tile([128, 1], F32, name=f"ri{t}", tag="ssq")
                nc.vector.reciprocal(rinv[:], std[:])
                x2 = xnp.tile([128, D], F16, name=f"xn{t}", tag="xn")
                nc.gpsimd.tensor_scalar_mul(x2[:], xt[t][:], rinv[:])
                xn.append(x2)

            # ---------------- transpose xn -> xnT [d, tok] ----------------
            xnT = []
            for dc in range(DC):
                xT = xntp.tile([128, N], F16, name=f"xnT{dc}", tag="xnT")
                for half in range(2):
                    tp = psa.tile([128, 1024], F16, name=f"tp{dc}{half}",
                                  tag="sim")
                    for j in range(8):
                        t = half * 8 + j
                        nc.tensor.transpose(
                            tp[:, j * 128:(j + 1) * 128],
                            xn[t][:, dc * 128:(dc + 1) * 128],
                            identf[:])
                    nc.vector.tensor_copy(
                        xT[:, half * 1024:(half + 1) * 1024], tp[:])
                xnT.append(xT)

            # ---------------- projections ----------------
            # kT/qT feature-major pair tiles [128 feats(2 heads), 2048 tok]
            def proj_fmajor(col0, name):
                tiles = []
                for i in range(2):
                    pt = kqp.tile([128, N], F16, name=f"{name}{i}", tag=name,
                                  bufs=2)
                    for half in range(2):
                        ps = psa.tile([128, 1024], F32, name=f"p{name}{i}{half}",
                                      tag="sim")
                        for tc2 in range(2):
                            tcol = half * 1024 + tc2 * 512
                            for dc in range(DC):
                                nc.tensor.matmul(
                                    ps[:, tc2 * 512:(tc2 + 1) * 512],
                                    w_sb[dc][:, col0 + i * 128:col0 + (i + 1) * 128],
                                    xnT[dc][:, tcol:tcol + 512],
                                    start=(dc == 0), stop=(dc == DC - 1))
                        nc.scalar.copy(
                            pt[:, half * 1024:(half + 1) * 1024], ps[:])
                    tiles.append(pt)
                return tiles

            kTp = proj_fmajor(HPC * DH, "kT")
            qTp = proj_fmajor(0, "qT")

            # v token-major, per head [128 k-part, 16 kc * 65] bf16 with a
            # ones column at slot 64 of each kc block (softmax denominator).
            vx = []
            for h in range(HPC):
                v = vxp.tile([128, KC * 65], BF16, name=f"vx{h}", tag="vx")
                nc.any.memset(
                    v[:].rearrange("p (kc c) -> p kc c", c=65)[:, :, 64:65],
                    1.0)
                vx.append(v)
            for t in range(NT):
                ps = psa.tile([128, 1024], F32, name=f"pv{t}", tag="sim")
                for dc in range(DC):
                    nc.tensor.matmul(
                        ps[:, 0:HPC * DH],
                        xnT[dc][:, t * 128:(t + 1) * 128],
                        w_sb[dc][:, 2 * HPC * DH:3 * HPC * DH],
                        start=(dc == 0), stop=(dc == DC - 1))
                for h in range(HPC):
                    nc.scalar.copy(
                        vx[h][:, t * 65:t * 65 + 64],
                        ps[:, h * 64:(h + 1) * 64])

            # ---------------- attention ----------------
            aout_tiles = {}  # (hp, qt) -> [128 q, 128 f] bf16 pair tile
            aoutT = []

            def attn_unit(h, qt):
                i, row = h // 2, (h % 2) * 64
                # q-major sim, keys split in two psum tiles
                sims = []
                for half in range(2):
                    ps = psa.tile([128, 1024], F32, name=f"s{h}{qt}{half}",
                                  tag="sim")
                    for kc2 in range(2):
                        kcol = half * 1024 + kc2 * 512
                        nc.tensor.matmul(
                            ps[:, kc2 * 512:(kc2 + 1) * 512],
                            qTp[i][row:row + 64, qt * 128:(qt + 1) * 128],
                            kTp[i][row:row + 64, kcol:kcol + 512],
                            start=True, stop=True)
                    sims.append(ps)
                # exact row max (free-axis reduce is DVE-only)
                sa = statsp.tile([128, 2], F32, name=f"sa{h}{qt}", tag="sa")
                nc.vector.tensor_reduce(sa[:, 0:1], sims[0][:],
                                        axis=mybir.AxisListType.X, op=ALU.max)
                nc.vector.tensor_reduce(sa[:, 1:2], sims[1][:],
                                        axis=mybir.AxisListType.X, op=ALU.max)
                negm = statsp.tile([128, 1], F32, name=f"nm{h}{qt}", tag="nm")
                nc.vector.tensor_reduce(negm[:], sa[:],
                                        axis=mybir.AxisListType.X,
                                        op=ALU.max, negate=True)
                # exp
                at = attnp.tile([128, N], BF16, name=f"at{h}{qt}", tag="at")
                for half in range(2):
                    nc.scalar.activation(
                        at[:, half * 1024:(half + 1) * 1024],
                        sims[half][:], AF.Exp, bias=negm[:])
                # keys-major transpose via DMA xbar
                atT = attntp.tile([128, KC * 128], BF16, name=f"atT{h}{qt}",
                                  tag="atT")
                nc.sync.dma_start_transpose(
                    atT[:].rearrange("p (kc q) -> p kc q", q=128), at[:])
                # AV with ones-column denominator
                av = psb.tile([128, 65], F32, name=f"av{h}{qt}", tag="av")
                atT3 = atT[:].rearrange("p (kc q) -> p kc q", q=128)
                for kc in range(KC):
                    nc.tensor.matmul(
                        av[:],
                        atT3[:, kc, :],
                        vx[h][:, kc * 65:(kc + 1) * 65],
                        start=(kc == 0), stop=(kc == KC - 1))
                # normalize into head-pair aout tile
                rs = statsp.tile([128, 1], F32, name=f"rs{h}{qt}", tag="rs")
                nc.vector.reciprocal(rs[:], av[:, 64:65])
                hp = h // 2
                if (hp, qt) not in aout_tiles:
                    aout_tiles[(hp, qt)] = aoutp.tile(
                        [128, 128], BF16, name=f"ao{hp}{qt}", tag="ao")
                nc.scalar.activation(
                    aout_tiles[(hp, qt)][:, (h % 2) * 64:(h % 2) * 64 + 64],
                    av[:, 0:64], AF.Copy, scale=rs[:])

            def aout_transpose(hp):
                aT = aouttp.tile([128, N], BF16, name=f"aoutT{hp}", tag="aT")
                for half in range(2):
                    tp = psa.tile([128, 1024], BF16, name=f"tpa{hp}{half}",
                                  tag="sim")
                    for j in range(8):
                        qt = half * 8 + j
                        nc.tensor.transpose(
                            tp[:, j * 128:(j + 1) * 128],
                            aout_tiles[(hp, qt)][:], identb[:])
                    nc.vector.tensor_copy(
                        aT[:, half * 1024:(half + 1) * 1024], tp[:])
                aoutT.append(aT)

            for h in range(HPC):
                for qt in range(QT):
                    attn_unit(h, qt)
                if h % 2 == 1:
                    aout_transpose(h // 2)

            # ---------------- output projection + reduce-scatter ----------
            def outproj(qt):
                ps = psa.tile([128, 1024], F32, name=f"po{qt}", tag="sim")
                for oc in range(2):
                    for hp in range(2):
                        nc.tensor.matmul(
                            ps[:, oc * 512:(oc + 1) * 512],
                            aoutT[hp][:, qt * 128:(qt + 1) * 128],
                            wout_sb[hp][:, oc * 512:(oc + 1) * 512],
                            start=(hp == 0), stop=(hp == 1))
                ot = osbp.tile([128, D], BF16, name=f"o{qt}", tag="o")
                nc.scalar.copy(ot[:], ps[:])
                nc.sync.dma_start(outp_d[qt * 128:(qt + 1) * 128, :], ot[:])

            for half in range(2):
                for j in range(QT // 2):
                    outproj(half * 8 + j)
                import os as _os
                if _os.environ.get("KERNEL_FAKE_COMM") == "1":
                    nc.sync.dma_start(
                        rsout_d[half * 256:(half + 1) * 256, :],
                        outp_d[half * 1024:half * 1024 + 256, :])
                else:
                    nc.gpsimd.collective_compute(
                        "ReduceScatter", ALU.add, replica_groups=rg,
                        ins=[outp_d[half * 1024:(half + 1) * 1024, :].opt()],
                        outs=[rsout_d[half * 256:(half + 1) * 256, :].opt()])
                nc.sync.dma_start(out_d[half * 256:(half + 1) * 256, :],
                                  rsout_d[half * 256:(half + 1) * 256, :])

    nc.finalize()
    return nc


_NC_CACHE = None


def kernel(x, mask, gamma, w_qkv, w_out):
    global _NC_CACHE
    x = np.asarray(x, dtype=np.float32)
    gamma = np.asarray(gamma, dtype=np.float32)
    w_qkv = np.asarray(w_qkv, dtype=np.float32)
    w_out = np.asarray(w_out, dtype=np.float32)

    # fold gamma (RMSNorm scale) and the x8 q-scale into w_qkv (exact in f32)
    w = w_qkv * gamma[:, None]
    w = np.concatenate([w[:, :D] * (DH ** 0.5), w[:, D:]], axis=1)

    if _NC_CACHE is None:
        _NC_CACHE = build_graph()
    nc = _NC_CACHE

    in_maps = []
    for c in range(NC_TOTAL):
        b, hg = divmod(c, GROUP)
        cs = slice(hg * HPC * DH, (hg + 1) * HPC * DH)
        wq = w[:, 0:D][:, cs]
        wk = w[:, D:2 * D][:, cs]
        wv = w[:, 2 * D:3 * D][:, cs]
        wc = np.ascontiguousarray(
            np.concatenate([wq, wk, wv], axis=1), dtype=np.float16)
        wo = np.ascontiguousarray(
            w_out[cs, :].astype(ml_dtypes.bfloat16))
        xs = np.ascontiguousarray(x[b], dtype=np.float16)
        in_maps.append({"x": xs, "w_qkv": wc, "w_out": wo})

    res = run_bass_kernel_spmd(nc, in_maps, core_ids=list(range(NC_TOTAL)))
    out = np.empty((B, N, D), dtype=np.float32)
    for c in range(NC_TOTAL):
        b, r = divmod(c, GROUP)
        o = np.asarray(res.results[c]["out"]).astype(np.float32)
        out[b, r * 256:(r + 1) * 256, :] = o[0:256]
        out[b, N // 2 + r * 256:N // 2 + (r + 1) * 256, :] = o[256:512]
    return out
